# revision 9
# baseline (speedup 1.0000x reference)
"""BERT self-attention on 8 Trainium2 NeuronCores.

Sharding: data-parallel over batch (B=8 -> 1 batch element per core).
Every core runs the same single-core Bass kernel on its own batch slice;
weights/mask are replicated. The final output is a host-side stack.

Per-core algorithm (S=1024, HID=1024, NH=16, HD=64), all matmuls bf16
with fp32 PSUM accumulation:

  xT = X^T (host-transposed, bf16)             [HID, S]
  Q^T = Wq^T @ X^T   (lhsT = Wq natural)       [HID, S]  (+bq per-partition)
  K^T = Wk^T @ X^T                             [HID, S]  (+bk per-partition)
  V   = X @ Wv       (lhsT = xT)               [S, HID]  (+bv broadcast)
  per chunk c (heads A=2c, B=2c+1 in partition halves of qT/kT):
    S^T = K_h @ Q_h^T  as PE ROW-TILED pairs: head A contracts over
          partitions 0:64 (tile (0,0)), head B over 64:128 ((64,0)).
          Interleaved A/B matmuls run CONCURRENTLY on the two row halves
          of the PE array (HW-verified ~1.9x: 116ns vs 218ns per 512-col
          matmul), so scores cost ~3.7us/chunk instead of ~6.9.
    P^T = exp(S^T/8 + mask[k])   (ScalarE; the 147us exp stream is the
          kernel's critical path -- everything else hides under it)
    ctx = P^T.T @ [V_h + bv | 1] (lhsT = P^T; ones column gives Z)
    out[:, h] = ctx[:, :64] * (1/Z)

The emission order is a hand-scheduled pipeline: QK(c0) runs during the
xT DMA fill (wq/wk column-block c0 is prefetched on side queues), so the
exp stream starts at ~18us; V-groups, next-chunk QK projections, and
previous-chunk ctx chains are spread between the exp-gated score tiles
of each chunk so the PE fills ScalarE's slack without ever head-blocking
the in-order tensor queue.
"""

import functools

import numpy as np
import ml_dtypes

B, S, HID = 8, 1024, 1024
NH, HD = 16, 64
P = 128
NCH = HID // P  # hid chunks (8)
NKT = S // P  # key tiles (8)
NQT = S // P  # query tiles (8)
VROW = NH * (HD + 1)  # 1040: per-seq-chunk V row: 16 x (64 V cols + ones col)
N_CORES = 8

SCALE = 1.0 / float(np.sqrt(HD))


@functools.lru_cache(maxsize=None)
def _build(has_bv: bool):
    import concourse.bass as bass
    import concourse.tile as tile
    from concourse import bacc, mybir
    from contextlib import ExitStack

    fp32 = mybir.dt.float32
    bf16 = mybir.dt.bfloat16
    EXP = mybir.ActivationFunctionType.Exp

    nc = bacc.Bacc("TRN2", target_bir_lowering=False)

    xT = nc.dram_tensor("xT", [HID, S], bf16, kind="ExternalInput")
    wq = nc.dram_tensor("wq", [HID, HID], bf16, kind="ExternalInput")
    wk = nc.dram_tensor("wk", [HID, HID], bf16, kind="ExternalInput")
    wv = nc.dram_tensor("wv", [HID, HID], bf16, kind="ExternalInput")
    bq = nc.dram_tensor("bq", [P, NCH], fp32, kind="ExternalInput")
    bk = nc.dram_tensor("bk", [P, NCH], fp32, kind="ExternalInput")
    bv = nc.dram_tensor("bv", [HID], fp32, kind="ExternalInput") if has_bv else None
    mask = nc.dram_tensor("mask", [P, NKT], fp32, kind="ExternalInput")
    out = nc.dram_tensor("out", [S, HID], fp32, kind="ExternalOutput")

    with tile.TileContext(nc) as tc, ExitStack() as ctx:
        persist = ctx.enter_context(tc.tile_pool(name="persist", bufs=1))
        misc = ctx.enter_context(tc.tile_pool(name="misc", bufs=8))
        pT_pool = ctx.enter_context(tc.tile_pool(name="pT", bufs=4))
        out_pool = ctx.enter_context(tc.tile_pool(name="out", bufs=2))
        qkv_ps = ctx.enter_context(tc.tile_pool(name="qkv_ps", bufs=2, space="PSUM"))
        sc_ps = ctx.enter_context(tc.tile_pool(name="sc_ps", bufs=2, space="PSUM"))
        cx_ps = ctx.enter_context(tc.tile_pool(name="cx_ps", bufs=2, space="PSUM"))

        # ---- persistent SBUF tensors ----
        xT_c = [persist.tile([P, S], bf16, name=f"xT{c}") for c in range(NCH)]
        wq_c = [persist.tile([P, HID], bf16, name=f"wq{c}") for c in range(NCH)]
        wk_c = [persist.tile([P, HID], bf16, name=f"wk{c}") for c in range(NCH)]
        wv_c = [persist.tile([P, HID], bf16, name=f"wv{c}") for c in range(NCH)]
        qT_sb = persist.tile([P, NCH, S], bf16)  # [p, hidout_chunk, seq]
        kT_sb = persist.tile([P, NCH, S], bf16)  # same layout (natural, no pad)
        v_sb = persist.tile([P, NKT, VROW], bf16)  # [p(seq), seq_chunk, 16*(64+1)]
        bq_sb = persist.tile([P, NCH], fp32)
        bk_sb = persist.tile([P, NCH], fp32)
        mask_sb = persist.tile([P, NKT], fp32)
        bv_sb = persist.tile([P, HID], fp32, name="bv_sb") if has_bv else None

        # ---- input DMAs ----
        # sync: smalls first (QK(c0)'s bias copies need them early!), then
        # xT/wv interleaved (the V phase's inputs, chunk-paced), then the
        # wq/wk column remainders.
        nc.sync.dma_start(out=bq_sb, in_=bq[:, :])
        nc.sync.dma_start(out=bk_sb, in_=bk[:, :])
        nc.sync.dma_start(out=mask_sb, in_=mask[:, :])
        if has_bv:
            bv_bcast = bass.AP(tensor=bv.tensor if hasattr(bv, "tensor") else bv,
                               offset=0, ap=[[0, P], [1, HID]])
            nc.sync.dma_start(out=bv_sb, in_=bv_bcast)
        for c in range(NCH):
            nc.sync.dma_start(out=xT_c[c], in_=xT[c * P:(c + 1) * P, :])
            nc.sync.dma_start(out=wv_c[c], in_=wv[c * P:(c + 1) * P, :])
        for c in range(NCH):
            nc.sync.dma_start(out=wq_c[c][:, P:], in_=wq[c * P:(c + 1) * P, P:])
        for c in range(NCH):
            nc.sync.dma_start(out=wk_c[c][:, P:], in_=wk[c * P:(c + 1) * P, P:])
        # side queues: the c0 column blocks of wq/wk (0.5MB total) land by
        # ~11us so QK(c0) can run during the xT fill
        for c in range(NCH):
            nc.scalar.dma_start(out=wq_c[c][:, 0:P], in_=wq[c * P:(c + 1) * P, 0:P])

        # ones columns for the softmax denominator (col 64 of each 65-wide
        # head block; V copies only overwrite cols 0..63). gpsimd is idle;
        # the wk c0 blocks follow it on the gpsimd queue (land ~13us, K(c0)
        # needs them from ~16).
        nc.gpsimd.memset(v_sb, 1.0)
        for c in range(NCH):
            nc.gpsimd.dma_start(out=wk_c[c][:, 0:P], in_=wk[c * P:(c + 1) * P, 0:P])

        # warmup matmuls: hold the HAM clock-gate at 8/8 through the fill
        wscr = persist.tile([P, 512], bf16, name="warm_scratch")
        nc.vector.memset(wscr, 0.5)
        for _ in range(20):
            wps = sc_ps.tile([P, S], fp32, name="score_psum")
            nc.tensor.matmul(
                wps[:, 0:512],
                lhsT=wscr[:, 0:P],
                rhs=wscr,
                start=True,
                stop=True,
            )

        # ---- emission helpers (in-order queues => this IS the schedule) ----
        def q_proj(c, half):
            ps = qkv_ps.tile([P, 512], fp32, name="qkv_psum")
            for kc in range(NCH):
                nc.tensor.matmul(
                    ps,
                    lhsT=wq_c[kc][:, c * P:(c + 1) * P],
                    rhs=xT_c[kc][:, half * 512:(half + 1) * 512],
                    start=(kc == 0),
                    stop=(kc == NCH - 1),
                )
            nc.vector.tensor_scalar_add(
                out=qT_sb[:, c, half * 512:(half + 1) * 512],
                in0=ps,
                scalar1=bq_sb[:, c:c + 1],
            )

        def k_proj(c, half):
            ps = qkv_ps.tile([P, 512], fp32, name="qkv_psum")
            for kc in range(NCH):
                nc.tensor.matmul(
                    ps,
                    lhsT=wk_c[kc][:, c * P:(c + 1) * P],
                    rhs=xT_c[kc][:, half * 512:(half + 1) * 512],
                    start=(kc == 0),
                    stop=(kc == NCH - 1),
                )
            nc.vector.tensor_scalar_add(
                out=kT_sb[:, c, half * 512:(half + 1) * 512],
                in0=ps,
                scalar1=bk_sb[:, c:c + 1],
            )

        def v_group(st, half):
            ps = qkv_ps.tile([P, 512], fp32, name="qkv_psum")
            for kc in range(NCH):
                nc.tensor.matmul(
                    ps,
                    lhsT=xT_c[kc][:, st * P:(st + 1) * P],
                    rhs=wv_c[kc][:, half * 512:(half + 1) * 512],
                    start=(kc == 0),
                    stop=(kc == NCH - 1),
                )
            dst = (
                v_sb[:, st, :]
                .rearrange("p (h x) -> p h x", x=HD + 1)[:, half * 8:(half + 1) * 8, 0:HD]
            )
            src = ps.rearrange("p (h x) -> p h x", x=HD)
            if has_bv:
                bvs = (
                    bv_sb[:, half * 512:(half + 1) * 512]
                    .rearrange("p (h x) -> p h x", x=HD)
                )
                nc.vector.tensor_add(out=dst, in0=src, in1=bvs)
            else:
                nc.vector.tensor_copy(out=dst, in_=src)

        pT_tiles = {}  # (c, sub) -> tile

        def sc_kt(c, kt):
            # row-tiled score pair for both heads of chunk c, key tile kt.
            # A/B matmuls interleave so consecutive queue entries hit
            # disjoint PE row groups and stream concurrently.
            psAB = [sc_ps.tile([P, S], fp32, name="score_psum") for _ in range(2)]
            for half in range(2):
                for sub in range(2):
                    po = 64 * sub
                    nc.tensor.matmul(
                        psAB[sub][:, half * 512:(half + 1) * 512],
                        lhsT=kT_sb[po:po + HD, c, kt * P:(kt + 1) * P],
                        rhs=qT_sb[po:po + HD, c, half * 512:(half + 1) * 512],
                        start=True,
                        stop=True,
                    )
            for sub in range(2):
                nc.scalar.activation(
                    out=pT_tiles[(c, sub)][:, kt, :],
                    in_=psAB[sub],
                    func=EXP,
                    bias=mask_sb[:, kt:kt + 1],
                    scale=SCALE,
                )

        def ctx_chain(c, sub, qt, head_out):
            h = 2 * c + sub
            pT_h = pT_tiles[(c, sub)]
            cps = cx_ps.tile([P, HD + 1], fp32, name="ctx_psum")
            for kc in range(NKT):
                nc.tensor.matmul(
                    cps,
                    lhsT=pT_h[:, kc, qt * P:(qt + 1) * P],
                    rhs=v_sb[:, kc, h * (HD + 1):(h + 1) * (HD + 1)],
                    start=(kc == 0),
                    stop=(kc == NKT - 1),
                )
            recip = misc.tile([P, 1], fp32, name="recip")
            nc.vector.reciprocal(recip, cps[:, HD:HD + 1])
            nc.vector.tensor_scalar_mul(
                out=head_out[:, qt, :],
                in0=cps[:, 0:HD],
                scalar1=recip,
            )

        def head_dma(c, sub, head_out):
            # one 3D-AP DMA for the whole head's [S, 64] output column block
            # (16 DMAs total instead of 128 -- the old per-qt pattern cost
            # ~0.6us each serialized on one queue, a ~10us kernel tail)
            h = 2 * c + sub
            dst = bass.AP(
                tensor=out.tensor if hasattr(out, "tensor") else out,
                offset=h * HD,
                ap=[[HID, P], [P * HID, NQT], [1, HD]],
            )
            eng = nc.sync if (h % 2 == 0) else nc.scalar
            eng.dma_start(out=dst, in_=head_out)

        def ctx_fillers(c):
            # 18 filler items: both heads' 8 ctx chains + 1 output DMA each
            items = []
            for sub in range(2):
                head_out = out_pool.tile([P, NQT, HD], fp32, name="head_out")
                for qt in range(NQT):
                    items.append(functools.partial(ctx_chain, c, sub, qt, head_out))
                items.append(functools.partial(head_dma, c, sub, head_out))
            return items

        def qk_fillers(c):
            return [functools.partial(q_proj, c, 0), functools.partial(q_proj, c, 1),
                    functools.partial(k_proj, c, 0), functools.partial(k_proj, c, 1)]

        def v_fillers(groups):
            return [functools.partial(v_group, st, half) for st, half in groups]

        VH0 = [(st, 0) for st in range(NKT)]  # heads 0-7  (chunks 0-3)
        VH1 = [(st, 1) for st in range(NKT)]  # heads 8-15 (chunks 4-7)

        # ---- QK(c0) during the DMA fill ----
        for f in qk_fillers(0):
            f()

        # Per-chunk filler schedule (see module docstring). Constraints:
        # QK(c) before chunk c's scores; all VH0 before ctx(c0); all VH1
        # before ctx(c4); ctx(c) right after chunk c+1's scores (pT slot
        # recycling with bufs=4).
        def emit_chunk(c, fillers):
            # allocate this chunk's pT tiles
            for sub in range(2):
                pT_tiles[(c, sub)] = pT_pool.tile([P, NKT, S], bf16, name="pT")
            n = len(fillers)
            done = 0
            for kt in range(NKT):
                sc_kt(c, kt)
                want = (kt + 1) * n // NKT
                while done < want:
                    fillers[done]()
                    done += 1

        # qk fillers lead each window so the next chunk's Q/K bias copies
        # reach the vector queue before the 32 ctx-norm ops (else the norm
        # backlog head-of-line-blocks the copies and the exp stream gaps
        # at every chunk boundary)
        emit_chunk(0, qk_fillers(1) + v_fillers(VH0[0:6]))
        emit_chunk(1, qk_fillers(2) + v_fillers(VH0[6:8]) + ctx_fillers(0))
        emit_chunk(2, qk_fillers(3) + ctx_fillers(1) + v_fillers(VH1[0:2]))
        emit_chunk(3, qk_fillers(4) + ctx_fillers(2) + v_fillers(VH1[2:5]))
        emit_chunk(4, qk_fillers(5) + ctx_fillers(3) + v_fillers(VH1[5:8]))
        emit_chunk(5, qk_fillers(6) + ctx_fillers(4))
        emit_chunk(6, qk_fillers(7) + ctx_fillers(5))
        emit_chunk(7, ctx_fillers(6))

        # anti-throttle: keep the PE clock at 8/8 through the final exps
        for _ in range(12):
            wps = sc_ps.tile([P, S], fp32, name="score_psum")
            nc.tensor.matmul(
                wps[:, 0:512],
                lhsT=wscr[:, 0:P],
                rhs=wscr,
                start=True,
                stop=True,
            )
        for f in ctx_fillers(7):
            f()

    nc.finalize()
    return nc


def _prep_inputs(inputs):
    bf16 = ml_dtypes.bfloat16
    hs = np.asarray(inputs["hidden_states"], dtype=np.float32)
    am = np.asarray(inputs["attention_mask"], dtype=np.float32)
    Wq = np.asarray(inputs["Wq"], dtype=np.float32)
    Wk = np.asarray(inputs["Wk"], dtype=np.float32)
    Wv = np.asarray(inputs["Wv"], dtype=np.float32)
    bq = np.asarray(inputs["bq"], dtype=np.float32)
    bk = np.asarray(inputs["bk"], dtype=np.float32)
    bv = np.asarray(inputs["bv"], dtype=np.float32)

    has_bv = bool(np.any(bv))

    wq_b = np.ascontiguousarray(Wq.astype(bf16))
    wk_b = np.ascontiguousarray(Wk.astype(bf16))
    wv_b = np.ascontiguousarray(Wv.astype(bf16))
    bq_c = np.ascontiguousarray(bq.reshape(NCH, P).T)
    bk_c = np.ascontiguousarray(bk.reshape(NCH, P).T)

    hs_b = hs.astype(bf16)
    in_maps = []
    for b in range(B):
        m = {
            "xT": np.ascontiguousarray(hs_b[b].T),
            "wq": wq_b,
            "wk": wk_b,
            "wv": wv_b,
            "bq": bq_c,
            "bk": bk_c,
            "mask": np.ascontiguousarray(am[b, 0, 0].reshape(NKT, P).T),
        }
        if has_bv:
            m["bv"] = bv
        in_maps.append(m)
    return in_maps, has_bv


def _run(inputs, trace=False, trace_cores=None):
    from concourse.bass_utils import run_bass_kernel_spmd

    in_maps, has_bv = _prep_inputs(inputs)
    nc = _build(has_bv)
    res = run_bass_kernel_spmd(
        nc, in_maps, core_ids=list(range(N_CORES)), trace=trace,
        trace_cores=trace_cores,
    )
    out = np.stack([np.asarray(r["out"], dtype=np.float32) for r in res.results])
    return out, res


def kernel(**inputs) -> np.ndarray:
    out, _ = _run(inputs, trace=False)
    return out
